# revision 9
# baseline (speedup 1.0000x reference)
"""Trainium2 Bass kernel for nn_CA1AttentionGate.

Computes, for full inputs (B=1, S=8192, H=1024, F=128, K=2):
    temporal = relu(t @ Wt1 + bt1) @ Wt2 + bt2          [K,F]
    mem      = dg_features + temporal                    [K,F]
    qmean    = query.mean(axis=1)                        [1,H]
    score_k  = tanh([mem_k ; qmean] @ Wa1 + ba1) @ Wa2 + ba2
    w_k      = sigmoid(score_k)
    g_k      = mem_k @ Wg + bg                           [K,H]
    row[s]   = (1/K) * sum_k w_k * (g_k . key[s])        [S]
    out      = broadcast(row) -> [1,1,S,S]

Sharding: sequence-parallel over the key/seq axis across 8 cores.  Every
row of the [S,S] output is the same vector, so each core computes only
its 1024-entry slice of that broadcast row from its key shard (the
sharding_hint's "slice of the broadcast row") and the host unshard step
expands the gathered row to the full output.  The only cross-core
quantity is qmean: each core reduces its query shard to per-chunk column
sums and a 4KB AllGather completes the mean (fallback variant replicates
the full query read).

Inputs are staged transposed ([H, shard]) and in bf16 so the query
column-sums are free-axis DVE reduces and the g.key matvec is a chain of
bf16 PE matmuls accumulating in PSUM; 1/K is folded into Wg/bg and 1/S
into the qmean rows of Wa1 on the host.
"""

import os

import numpy as np

SEQ = 8192
H = 1024
F = 128
K = 2
NCORES = 8
SHARD = SEQ // NCORES  # 1024
NCH = H // 128  # 8 h-chunks of 128

# packed f32 constant tensor [128, NP_] column layout
_C_WT1 = 0
_C_BT1 = 1
_C_TS = 2
_C_WT2 = 4
_C_BT2 = 132
_C_DGT = 133
_C_BA1 = 135
_C_BA2 = 136
_C_BGT = 137
_C_WA2 = 145
_C_WA1M = 146
_C_WA1Q = 274
_C_WG = 274 + H
NP_ = _C_WG + H

_PROG_CACHE = {}


def _build(use_collective: bool):
    import concourse.bacc as bacc
    import concourse.tile as tile
    from concourse import mybir
    from concourse.tile_rust import add_dep_helper

    AF = mybir.ActivationFunctionType
    ALU = mybir.AluOpType
    f32 = mybir.dt.float32
    bf16 = mybir.dt.bfloat16

    nc = bacc.Bacc(
        "TRN2",
        target_bir_lowering=False,
        debug=False,
        num_devices=NCORES,
    )

    qcols = SHARD if use_collective else SEQ
    qs = nc.dram_tensor("qs", [H, qcols], bf16, kind="ExternalInput").ap()
    ks = nc.dram_tensor("ks", [H, SHARD], bf16, kind="ExternalInput").ap()
    Pc = nc.dram_tensor("P", [128, NP_], f32, kind="ExternalInput").ap()
    out = nc.dram_tensor("out", [1, SHARD], f32, kind="ExternalOutput").ap()

    with tile.TileContext(nc) as tc:
        with (
            tc.tile_pool(name="consts", bufs=1) as cp,
            tc.tile_pool(name="work", bufs=1) as wp,
            tc.tile_pool(name="qstream", bufs=NCH if use_collective else 3) as qp,
            tc.tile_pool(name="kstream", bufs=NCH) as kp,
            tc.tile_pool(name="ps_small", bufs=2, space="PSUM") as pps,
            tc.tile_pool(name="ps_keep", bufs=1, space="PSUM") as ppk,
            tc.tile_pool(name="ps_big", bufs=1, space="PSUM") as ppb,
            tc.tile_pool(name="dram", bufs=1, space="DRAM") as dp,
        ):
            # ---- query chunk loads get the wire first: they feed the
            # qmean partial sums and the collective, the head of the
            # critical path.  qtile[c][p, s] = q[s, c*128+p].
            qv = qs.rearrange("(c p) s -> c p s", p=128)
            qtiles, q_insts = [], []
            for c in range(NCH):
                qt = qp.tile([128, qcols], bf16, tag="qt")
                q_insts.append(nc.sync.dma_start(qt, qv[c]))
                qtiles.append(qt)

            # warm the ACT tables NOW: the loads have no data deps and
            # must not steal engine time mid accum-chain or on the tail
            warm1 = wp.tile([1, 1], f32, tag="w1")
            nc.scalar.activation(warm1, qtiles[0][0:1, 0:1], AF.Tanh)
            warm2 = wp.tile([1, 1], f32, tag="w2")
            nc.scalar.activation(warm2, qtiles[0][0:1, 0:1], AF.Sigmoid)

            # ---- per-chunk query column sums, pipelined with the loads;
            # each chunk is split between DVE (left half, reduce) and ACT
            # (right half, copy+accum) so the per-chunk latency after the
            # last tile lands is one half-tile op, not a full-tile one:
            # qmTp[p, c] = sum_s q[s, c*128+p]  (f32 accumulate)
            qh = qcols // 2
            qmTpL = wp.tile([128, NCH], f32, tag="qmTpL")
            qmTpR = wp.tile([128, NCH], f32, tag="qmTpR")
            junk = wp.tile([128, qh], bf16, tag="junk")
            for c in range(NCH):
                nc.vector.tensor_reduce(
                    qmTpL[:, c : c + 1],
                    qtiles[c][:, 0:qh],
                    axis=mybir.AxisListType.X,
                    op=ALU.add,
                )
                nc.scalar.activation(
                    junk,
                    qtiles[c][:, qh:qcols],
                    AF.Copy,
                    accum_out=qmTpR[:, c : c + 1],
                )
            qmTp = wp.tile([128, NCH], f32, tag="qmTp")
            nc.vector.tensor_add(qmTp, qmTpL, qmTpR)

            cc_inst = None
            if use_collective:
                cc_in = dp.tile([128, NCH], f32)
                cc_out = dp.tile([NCORES, 128 * NCH], f32)
                cc_inst = nc.scalar.dma_start(cc_in, qmTp)
                nc.gpsimd.collective_compute(
                    "AllGather",
                    ALU.bypass,
                    replica_groups=[list(range(NCORES))],
                    ins=[cc_in.opt()],
                    outs=[cc_out.opt()],
                )

            # ---- packed constants: one DMA, kept off the wire until the
            # tiny collective input is out (it feeds only the k-side work
            # that hides under the collective)
            P = cp.tile([128, NP_], f32)
            p_inst = nc.scalar.dma_start(P, Pc)
            add_dep_helper(
                p_inst.ins,
                (cc_inst if cc_inst is not None else q_insts[-1]).ins,
                reason="consts after cc_in",
            )
            Wt1T = P[0:32, _C_WT1 : _C_WT1 + 1]
            bt1T = P[0:32, _C_BT1 : _C_BT1 + 1]
            tb = P[0:32, _C_TS : _C_TS + K]
            Wt2 = P[0:32, _C_WT2 : _C_WT2 + F]
            bt2T = P[:, _C_BT2 : _C_BT2 + 1]
            dgT = P[:, _C_DGT : _C_DGT + K]
            ba1T = P[:, _C_BA1 : _C_BA1 + 1]
            ba2c = P[0:K, _C_BA2 : _C_BA2 + 1]
            bgT = P[:, _C_BGT : _C_BGT + NCH]
            Wa2c = P[:, _C_WA2 : _C_WA2 + 1]
            Wa1m = P[:, _C_WA1M : _C_WA1M + F]

            # ---- key stream; ordered after cc_in so the tiny collective
            # input is not stuck behind 2MB of key reads
            kv = ks.rearrange("(c p) s -> c p s", p=128)
            ktiles = []
            for c in range(NCH):
                kt = kp.tile([128, SHARD], bf16, tag="kt")
                ki = nc.sync.dma_start(kt, kv[c])
                add_dep_helper(
                    ki.ins,
                    (cc_inst if cc_inst is not None else q_insts[-1]).ins,
                    reason="keys after cc_in",
                )
                ktiles.append(kt)

            # ---- temporal MLP -> memT [F, K] (f32) ----
            h1T = wp.tile([32, K], f32, tag="h1T")
            nc.vector.tensor_scalar_mul(h1T, tb, Wt1T)
            nc.vector.tensor_scalar_add(h1T, h1T, bt1T)
            nc.vector.tensor_relu(h1T, h1T)
            tT_ps = pps.tile([F, K], f32, tag="tmp")
            nc.tensor.matmul(tT_ps, lhsT=Wt2, rhs=h1T, start=True, stop=True)
            memT = wp.tile([F, K], f32, tag="memT")
            nc.scalar.activation(memT, tT_ps, AF.Identity, bias=bt2T, scale=1.0)
            nc.vector.tensor_add(memT, memT, dgT)

            # ---- mem half of the scorer (pre-collective) ----
            haT_ps = ppk.tile([F, K], f32, tag="haT")
            nc.tensor.matmul(haT_ps, lhsT=Wa1m, rhs=memT, start=True, stop=True)

            # ---- gate columns gT[c][p, k] = g_k[c*128+p] in bf16
            # (1/K and bg folded on the host)
            gt_bf = []
            for c in range(NCH):
                g_ps = pps.tile([F, K], f32, tag="gt")
                nc.tensor.matmul(
                    g_ps,
                    lhsT=P[:, _C_WG + c * 128 : _C_WG + (c + 1) * 128],
                    rhs=memT,
                    start=True,
                    stop=True,
                )
                gb = wp.tile([F, K], bf16, tag=f"gb{c}")
                nc.vector.tensor_scalar_add(gb, g_ps, bgT[:, c : c + 1])
                gt_bf.append(gb)

            # ---- matvec: row_ps[k, s] = g_k . key[s], chunked over h ----
            row_ps = ppb.tile([K, SHARD], f32, tag="big")
            for half in range(2):
                sl = slice(half * 512, (half + 1) * 512)
                for c in range(NCH):
                    nc.tensor.matmul(
                        row_ps[:, sl],
                        lhsT=gt_bf[c],
                        rhs=ktiles[c][:, sl],
                        start=(c == 0),
                        stop=(c == NCH - 1),
                    )
            row_sb = wp.tile([K, SHARD], bf16, tag="row")
            nc.scalar.copy(row_sb, row_ps)

            # ---- post-collective: qmT = sum of per-core partials ----
            if use_collective:
                qmTd8 = wp.tile([128, NCORES, NCH], f32, tag="qmTd8")
                nc.sync.dma_start(
                    qmTd8, cc_out[:, :].rearrange("d (p c) -> p d c", c=NCH)
                )
                qmT = wp.tile([128, NCH], f32, tag="qmT")
                nc.vector.tensor_reduce(
                    qmT,
                    qmTd8.rearrange("p d c -> p c d"),
                    axis=mybir.AxisListType.X,
                    op=ALU.add,
                )
            else:
                qmT = qmTp

            # ---- hq[f] = (qmean @ Wa1q)[f]  (1/S folded into Wa1q) ----
            hq_ps = ppk.tile([F, 1], f32, tag="hq")
            for c in range(NCH):
                nc.tensor.matmul(
                    hq_ps,
                    lhsT=P[:, _C_WA1Q + c * 128 : _C_WA1Q + (c + 1) * 128],
                    rhs=qmT[:, c : c + 1],
                    start=(c == 0),
                    stop=(c == NCH - 1),
                )
            hq_sb = wp.tile([F, 1], f32, tag="hq_sb")
            nc.vector.tensor_scalar_add(hq_sb, hq_ps, ba1T)

            # ---- scorer tail: tanh, score, sigmoid ----
            aT = wp.tile([F, K], f32, tag="aT")
            nc.scalar.activation(aT, haT_ps, AF.Tanh, bias=hq_sb, scale=1.0)
            score_ps = pps.tile([K, 1], f32, tag="tmp")
            nc.tensor.matmul(score_ps, lhsT=aT, rhs=Wa2c, start=True, stop=True)
            wcol = wp.tile([K, 1], bf16, tag="wcol")
            nc.scalar.activation(wcol, score_ps, AF.Sigmoid, bias=ba2c, scale=1.0)

            # ---- combine anchors: o_row = wcol^T @ row  (1/K in row);
            # the PSUM->SBUF copy of each half overlaps the other half's
            # matmul (ACT vs PE)
            orow_ps = ppb.tile([K, SHARD], f32, tag="big")
            orow_sb = wp.tile([1, SHARD], f32, tag="orow")
            for half in range(2):
                sl = slice(half * 512, (half + 1) * 512)
                nc.tensor.matmul(
                    orow_ps[0:1, sl],
                    lhsT=wcol,
                    rhs=row_sb[:, sl],
                    start=True,
                    stop=True,
                )
                nc.scalar.copy(orow_sb[:, sl], orow_ps[0:1, sl])
                # per-half store: the first half's DMA overlaps the
                # second half's combine+copy
                nc.sync.dma_start(out[:, sl], orow_sb[:, sl])

    nc.compile()
    return nc


def _get_prog(use_collective: bool):
    key = bool(use_collective)
    if key not in _PROG_CACHE:
        _PROG_CACHE[key] = _build(key)
    return _PROG_CACHE[key]


def _pack_consts(inputs) -> np.ndarray:
    f = lambda name: np.asarray(inputs[name], np.float32)
    P = np.zeros((128, NP_), np.float32)
    P[0:32, _C_WT1] = f("Wt1")[0]
    P[0:32, _C_BT1] = f("bt1")
    P[0:32, _C_TS : _C_TS + K] = np.broadcast_to(f("timestamps")[None, :], (32, K))
    P[0:32, _C_WT2 : _C_WT2 + F] = f("Wt2")
    P[:, _C_BT2] = f("bt2")
    P[:, _C_DGT : _C_DGT + K] = f("dg_features").T
    P[:, _C_BA1] = f("ba1")
    P[0:K, _C_BA2] = f("ba2")[0]
    P[:, _C_BGT : _C_BGT + NCH] = (f("bg") * (1.0 / K)).reshape(NCH, 128).T
    P[:, _C_WA2] = f("Wa2")[:, 0]
    P[:, _C_WA1M : _C_WA1M + F] = f("Wa1")[0:F, :]
    P[:, _C_WA1Q : _C_WA1Q + H] = (
        (f("Wa1")[F:, :] * (1.0 / SEQ))
        .reshape(NCH, 128, F)
        .transpose(1, 0, 2)
        .reshape(128, NCH * F)
    )
    P[:, _C_WG : _C_WG + H] = f("Wg") * (1.0 / K)
    return np.ascontiguousarray(P)


def _make_in_maps(inputs, use_collective: bool):
    import ml_dtypes

    bf16 = ml_dtypes.bfloat16
    q = np.asarray(inputs["query"], np.float32)[0]  # [S,H]
    k = np.asarray(inputs["key"], np.float32)[0]  # [S,H]
    P = _pack_consts(inputs)
    qb = q.astype(bf16)
    kb = k.astype(bf16)
    if not use_collective:
        q_full = np.ascontiguousarray(qb.T)  # [H, S]
    in_maps = []
    for d in range(NCORES):
        sl = slice(d * SHARD, (d + 1) * SHARD)
        m = {
            "P": P,
            "ks": np.ascontiguousarray(kb[sl].T),  # [H, SHARD]
            "qs": (
                np.ascontiguousarray(qb[sl].T) if use_collective else q_full
            ),
        }
        in_maps.append(m)
    return in_maps


def _run(inputs, use_collective: bool, trace: bool = False):
    from concourse.bass_utils import run_bass_kernel_spmd

    nc = _get_prog(use_collective)
    in_maps = _make_in_maps(inputs, use_collective)
    res = run_bass_kernel_spmd(
        nc, in_maps, core_ids=list(range(NCORES)), trace=trace
    )
    row = np.empty((SEQ,), np.float32)
    for d in range(NCORES):
        row[d * SHARD : (d + 1) * SHARD] = res.results[d]["out"][0]
    # every row of the [S, S] output is the same gate row
    full = np.empty((1, 1, SEQ, SEQ), np.float32)
    full[0, 0, :, :] = row[None, :]
    return full, res


def kernel(**inputs) -> np.ndarray:
    use_collective = os.environ.get("CA1_NO_COLLECTIVE", "0") != "1"
    try:
        full, _ = _run(inputs, use_collective)
        return full
    except Exception:
        if not use_collective:
            raise
        # fall back to the zero-communication variant (replicated query)
        _PROG_CACHE.pop(True, None)
        full, _ = _run(inputs, False)
        return full


# revision 10
# speedup vs baseline: 1.0438x; 1.0438x over previous
"""Trainium2 Bass kernel for nn_CA1AttentionGate.

Computes, for full inputs (B=1, S=8192, H=1024, F=128, K=2):
    temporal = relu(t @ Wt1 + bt1) @ Wt2 + bt2          [K,F]
    mem      = dg_features + temporal                    [K,F]
    qmean    = query.mean(axis=1)                        [1,H]
    score_k  = tanh([mem_k ; qmean] @ Wa1 + ba1) @ Wa2 + ba2
    w_k      = sigmoid(score_k)
    g_k      = mem_k @ Wg + bg                           [K,H]
    row[s]   = (1/K) * sum_k w_k * (g_k . key[s])        [S]
    out      = broadcast(row) -> [1,1,S,S]

Sharding: sequence-parallel over the key/seq axis across 8 cores.  Every
row of the [S,S] output is the same vector, so each core computes only
its 1024-entry slice of that broadcast row from its key shard (the
sharding_hint's "slice of the broadcast row") and the host unshard step
expands the gathered row to the full output.  The only cross-core
quantity is qmean: each core reduces its query shard to per-chunk column
sums and a 4KB AllGather completes the mean (fallback variant replicates
the full query read).

Inputs are staged transposed ([H, shard]) and in bf16 so the query
column-sums are free-axis DVE reduces and the g.key matvec is a chain of
bf16 PE matmuls accumulating in PSUM; 1/K is folded into Wg/bg and 1/S
into the qmean rows of Wa1 on the host.
"""

import os

import numpy as np

SEQ = 8192
H = 1024
F = 128
K = 2
NCORES = 8
SHARD = SEQ // NCORES  # 1024
NCH = H // 128  # 8 h-chunks of 128

# packed f32 constant tensor [128, NP_] column layout
_C_WT1 = 0
_C_BT1 = 1
_C_TS = 2
_C_WT2 = 4
_C_BT2 = 132
_C_DGT = 133
_C_BA1 = 135
_C_BA2 = 136
_C_BGT = 137
_C_WA2 = 145
_C_WA1M = 146
_C_WA1Q = 274
_C_WG = 274 + H
NP_ = _C_WG + H

_PROG_CACHE = {}


def _build(use_collective: bool):
    import concourse.bacc as bacc
    import concourse.tile as tile
    from concourse import mybir
    from concourse.tile_rust import add_dep_helper

    AF = mybir.ActivationFunctionType
    ALU = mybir.AluOpType
    f32 = mybir.dt.float32
    bf16 = mybir.dt.bfloat16

    nc = bacc.Bacc(
        "TRN2",
        target_bir_lowering=False,
        debug=False,
        num_devices=NCORES,
    )

    qcols = SHARD if use_collective else SEQ
    qs = nc.dram_tensor("qs", [H, qcols], bf16, kind="ExternalInput").ap()
    ks = nc.dram_tensor("ks", [H, SHARD], bf16, kind="ExternalInput").ap()
    Pc = nc.dram_tensor("P", [128, NP_], f32, kind="ExternalInput").ap()
    out = nc.dram_tensor("out", [1, SHARD], f32, kind="ExternalOutput").ap()

    with tile.TileContext(nc) as tc:
        with (
            tc.tile_pool(name="consts", bufs=1) as cp,
            tc.tile_pool(name="work", bufs=1) as wp,
            tc.tile_pool(name="qstream", bufs=NCH if use_collective else 3) as qp,
            tc.tile_pool(name="kstream", bufs=NCH) as kp,
            tc.tile_pool(name="ps_small", bufs=2, space="PSUM") as pps,
            tc.tile_pool(name="ps_keep", bufs=1, space="PSUM") as ppk,
            tc.tile_pool(name="ps_big", bufs=1, space="PSUM") as ppb,
            tc.tile_pool(name="dram", bufs=1, space="DRAM") as dp,
        ):
            # ---- query chunk loads get the wire first: they feed the
            # qmean partial sums and the collective, the head of the
            # critical path.  qtile[c][p, s] = q[s, c*128+p].
            qv = qs.rearrange("(c p) s -> c p s", p=128)
            qtiles, q_insts = [], []
            for c in range(NCH):
                qt = qp.tile([128, qcols], bf16, tag="qt")
                q_insts.append(nc.sync.dma_start(qt, qv[c]))
                qtiles.append(qt)

            # warm the ACT tables NOW against a dep-free memset tile: the
            # loads must not steal ACT time mid accum-chain or on the tail
            wsrc = wp.tile([1, 1], f32, tag="wsrc")
            nc.vector.memset(wsrc, 0.0)
            warm1 = wp.tile([1, 1], f32, tag="w1")
            nc.scalar.activation(warm1, wsrc, AF.Tanh)
            warm2 = wp.tile([1, 1], f32, tag="w2")
            nc.scalar.activation(warm2, wsrc, AF.Sigmoid)

            # ---- per-chunk query column sums, pipelined with the loads
            # and split across DVE (even chunks, reduce) and ACT (odd
            # chunks, copy+accum) so neither engine chain lags the
            # arriving tiles: qmTp[p, c] = sum_s q[s, c*128+p]
            qmTp = wp.tile([128, NCH], f32, tag="qmTp")
            junk = wp.tile([128, qcols], bf16, tag="junk")
            for c in range(NCH):
                if c % 2 == 0:
                    nc.vector.tensor_reduce(
                        qmTp[:, c : c + 1],
                        qtiles[c],
                        axis=mybir.AxisListType.X,
                        op=ALU.add,
                    )
                else:
                    nc.scalar.activation(
                        junk,
                        qtiles[c],
                        AF.Copy,
                        accum_out=qmTp[:, c : c + 1],
                    )

            cc_inst = None
            if use_collective:
                cc_in = dp.tile([128, NCH], f32)
                cc_out = dp.tile([NCORES, 128 * NCH], f32)
                cc_inst = nc.scalar.dma_start(cc_in, qmTp)
                nc.gpsimd.collective_compute(
                    "AllGather",
                    ALU.bypass,
                    replica_groups=[list(range(NCORES))],
                    ins=[cc_in.opt()],
                    outs=[cc_out.opt()],
                )

            # ---- packed constants: one DMA, kept off the wire until the
            # tiny collective input is out (it feeds only the k-side work
            # that hides under the collective)
            P = cp.tile([128, NP_], f32)
            p_inst = nc.scalar.dma_start(P, Pc)
            add_dep_helper(
                p_inst.ins,
                (cc_inst if cc_inst is not None else q_insts[-1]).ins,
                reason="consts after cc_in",
            )
            Wt1T = P[0:32, _C_WT1 : _C_WT1 + 1]
            bt1T = P[0:32, _C_BT1 : _C_BT1 + 1]
            tb = P[0:32, _C_TS : _C_TS + K]
            Wt2 = P[0:32, _C_WT2 : _C_WT2 + F]
            bt2T = P[:, _C_BT2 : _C_BT2 + 1]
            dgT = P[:, _C_DGT : _C_DGT + K]
            ba1T = P[:, _C_BA1 : _C_BA1 + 1]
            ba2c = P[0:K, _C_BA2 : _C_BA2 + 1]
            bgT = P[:, _C_BGT : _C_BGT + NCH]
            Wa2c = P[:, _C_WA2 : _C_WA2 + 1]
            Wa1m = P[:, _C_WA1M : _C_WA1M + F]

            # ---- key stream; ordered after cc_in so the tiny collective
            # input is not stuck behind 2MB of key reads
            kv = ks.rearrange("(c p) s -> c p s", p=128)
            ktiles = []
            for c in range(NCH):
                kt = kp.tile([128, SHARD], bf16, tag="kt")
                ki = nc.sync.dma_start(kt, kv[c])
                add_dep_helper(
                    ki.ins,
                    (cc_inst if cc_inst is not None else q_insts[-1]).ins,
                    reason="keys after cc_in",
                )
                ktiles.append(kt)

            # ---- temporal MLP -> memT [F, K] (f32) ----
            h1T = wp.tile([32, K], f32, tag="h1T")
            nc.vector.tensor_scalar_mul(h1T, tb, Wt1T)
            nc.vector.tensor_scalar_add(h1T, h1T, bt1T)
            nc.vector.tensor_relu(h1T, h1T)
            tT_ps = pps.tile([F, K], f32, tag="tmp")
            nc.tensor.matmul(tT_ps, lhsT=Wt2, rhs=h1T, start=True, stop=True)
            memT = wp.tile([F, K], f32, tag="memT")
            nc.scalar.activation(memT, tT_ps, AF.Identity, bias=bt2T, scale=1.0)
            nc.vector.tensor_add(memT, memT, dgT)

            # ---- mem half of the scorer (pre-collective) ----
            haT_ps = ppk.tile([F, K], f32, tag="haT")
            nc.tensor.matmul(haT_ps, lhsT=Wa1m, rhs=memT, start=True, stop=True)

            # ---- gate columns gT[c][p, k] = g_k[c*128+p] in bf16
            # (1/K and bg folded on the host)
            gt_bf = []
            for c in range(NCH):
                g_ps = pps.tile([F, K], f32, tag="gt")
                nc.tensor.matmul(
                    g_ps,
                    lhsT=P[:, _C_WG + c * 128 : _C_WG + (c + 1) * 128],
                    rhs=memT,
                    start=True,
                    stop=True,
                )
                gb = wp.tile([F, K], bf16, tag=f"gb{c}")
                nc.vector.tensor_scalar_add(gb, g_ps, bgT[:, c : c + 1])
                gt_bf.append(gb)

            # ---- matvec: row_ps[k, s] = g_k . key[s], chunked over h ----
            row_ps = ppb.tile([K, SHARD], f32, tag="big")
            for half in range(2):
                sl = slice(half * 512, (half + 1) * 512)
                for c in range(NCH):
                    nc.tensor.matmul(
                        row_ps[:, sl],
                        lhsT=gt_bf[c],
                        rhs=ktiles[c][:, sl],
                        start=(c == 0),
                        stop=(c == NCH - 1),
                    )
            row_sb = wp.tile([K, SHARD], bf16, tag="row")
            nc.scalar.copy(row_sb, row_ps)

            # ---- post-collective: qmT = sum of per-core partials ----
            if use_collective:
                qmTd8 = wp.tile([128, NCORES, NCH], f32, tag="qmTd8")
                nc.sync.dma_start(
                    qmTd8, cc_out[:, :].rearrange("d (p c) -> p d c", c=NCH)
                )
                qmT = wp.tile([128, NCH], f32, tag="qmT")
                nc.vector.tensor_reduce(
                    qmT,
                    qmTd8.rearrange("p d c -> p c d"),
                    axis=mybir.AxisListType.X,
                    op=ALU.add,
                )
            else:
                qmT = qmTp

            # ---- hq[f] = (qmean @ Wa1q)[f]  (1/S folded into Wa1q) ----
            hq_ps = ppk.tile([F, 1], f32, tag="hq")
            for c in range(NCH):
                nc.tensor.matmul(
                    hq_ps,
                    lhsT=P[:, _C_WA1Q + c * 128 : _C_WA1Q + (c + 1) * 128],
                    rhs=qmT[:, c : c + 1],
                    start=(c == 0),
                    stop=(c == NCH - 1),
                )
            hq_sb = wp.tile([F, 1], f32, tag="hq_sb")
            nc.vector.tensor_scalar_add(hq_sb, hq_ps, ba1T)

            # ---- scorer tail: tanh, score, sigmoid ----
            aT = wp.tile([F, K], f32, tag="aT")
            nc.scalar.activation(aT, haT_ps, AF.Tanh, bias=hq_sb, scale=1.0)
            score_ps = pps.tile([K, 1], f32, tag="tmp")
            nc.tensor.matmul(score_ps, lhsT=aT, rhs=Wa2c, start=True, stop=True)
            wcol = wp.tile([K, 1], bf16, tag="wcol")
            nc.scalar.activation(wcol, score_ps, AF.Sigmoid, bias=ba2c, scale=1.0)

            # ---- combine anchors: o_row = wcol^T @ row  (1/K in row);
            # the PSUM->SBUF copy of each half overlaps the other half's
            # matmul (ACT vs PE)
            orow_ps = ppb.tile([K, SHARD], f32, tag="big")
            orow_sb = wp.tile([1, SHARD], f32, tag="orow")
            for half in range(2):
                sl = slice(half * 512, (half + 1) * 512)
                nc.tensor.matmul(
                    orow_ps[0:1, sl],
                    lhsT=wcol,
                    rhs=row_sb[:, sl],
                    start=True,
                    stop=True,
                )
                nc.scalar.copy(orow_sb[:, sl], orow_ps[0:1, sl])
                # per-half store: the first half's DMA overlaps the
                # second half's combine+copy
                nc.sync.dma_start(out[:, sl], orow_sb[:, sl])

    nc.compile()
    return nc


def _get_prog(use_collective: bool):
    key = bool(use_collective)
    if key not in _PROG_CACHE:
        _PROG_CACHE[key] = _build(key)
    return _PROG_CACHE[key]


def _pack_consts(inputs) -> np.ndarray:
    f = lambda name: np.asarray(inputs[name], np.float32)
    P = np.zeros((128, NP_), np.float32)
    P[0:32, _C_WT1] = f("Wt1")[0]
    P[0:32, _C_BT1] = f("bt1")
    P[0:32, _C_TS : _C_TS + K] = np.broadcast_to(f("timestamps")[None, :], (32, K))
    P[0:32, _C_WT2 : _C_WT2 + F] = f("Wt2")
    P[:, _C_BT2] = f("bt2")
    P[:, _C_DGT : _C_DGT + K] = f("dg_features").T
    P[:, _C_BA1] = f("ba1")
    P[0:K, _C_BA2] = f("ba2")[0]
    P[:, _C_BGT : _C_BGT + NCH] = (f("bg") * (1.0 / K)).reshape(NCH, 128).T
    P[:, _C_WA2] = f("Wa2")[:, 0]
    P[:, _C_WA1M : _C_WA1M + F] = f("Wa1")[0:F, :]
    P[:, _C_WA1Q : _C_WA1Q + H] = (
        (f("Wa1")[F:, :] * (1.0 / SEQ))
        .reshape(NCH, 128, F)
        .transpose(1, 0, 2)
        .reshape(128, NCH * F)
    )
    P[:, _C_WG : _C_WG + H] = f("Wg") * (1.0 / K)
    return np.ascontiguousarray(P)


def _make_in_maps(inputs, use_collective: bool):
    import ml_dtypes

    bf16 = ml_dtypes.bfloat16
    q = np.asarray(inputs["query"], np.float32)[0]  # [S,H]
    k = np.asarray(inputs["key"], np.float32)[0]  # [S,H]
    P = _pack_consts(inputs)
    qb = q.astype(bf16)
    kb = k.astype(bf16)
    if not use_collective:
        q_full = np.ascontiguousarray(qb.T)  # [H, S]
    in_maps = []
    for d in range(NCORES):
        sl = slice(d * SHARD, (d + 1) * SHARD)
        m = {
            "P": P,
            "ks": np.ascontiguousarray(kb[sl].T),  # [H, SHARD]
            "qs": (
                np.ascontiguousarray(qb[sl].T) if use_collective else q_full
            ),
        }
        in_maps.append(m)
    return in_maps


def _run(inputs, use_collective: bool, trace: bool = False):
    from concourse.bass_utils import run_bass_kernel_spmd

    nc = _get_prog(use_collective)
    in_maps = _make_in_maps(inputs, use_collective)
    res = run_bass_kernel_spmd(
        nc, in_maps, core_ids=list(range(NCORES)), trace=trace
    )
    row = np.empty((SEQ,), np.float32)
    for d in range(NCORES):
        row[d * SHARD : (d + 1) * SHARD] = res.results[d]["out"][0]
    # every row of the [S, S] output is the same gate row
    full = np.empty((1, 1, SEQ, SEQ), np.float32)
    full[0, 0, :, :] = row[None, :]
    return full, res


def kernel(**inputs) -> np.ndarray:
    use_collective = os.environ.get("CA1_NO_COLLECTIVE", "0") != "1"
    try:
        full, _ = _run(inputs, use_collective)
        return full
    except Exception:
        if not use_collective:
            raise
        # fall back to the zero-communication variant (replicated query)
        _PROG_CACHE.pop(True, None)
        full, _ = _run(inputs, False)
        return full


# revision 15
# speedup vs baseline: 1.0694x; 1.0246x over previous
"""Trainium2 Bass kernel for nn_CA1AttentionGate.

Computes, for full inputs (B=1, S=8192, H=1024, F=128, K=2):
    temporal = relu(t @ Wt1 + bt1) @ Wt2 + bt2          [K,F]
    mem      = dg_features + temporal                    [K,F]
    qmean    = query.mean(axis=1)                        [1,H]
    score_k  = tanh([mem_k ; qmean] @ Wa1 + ba1) @ Wa2 + ba2
    w_k      = sigmoid(score_k)
    g_k      = mem_k @ Wg + bg                           [K,H]
    row[s]   = (1/K) * sum_k w_k * (g_k . key[s])        [S]
    out      = broadcast(row) -> [1,1,S,S]

Sharding: sequence-parallel over the key/seq axis across 8 cores.  Every
row of the [S,S] output is the same vector, so each core computes only
its 1024-entry slice of that broadcast row from its key shard (the
sharding_hint's "slice of the broadcast row") and the host unshard step
expands the gathered row to the full output.  The only cross-core
quantity is qmean: each core reduces its query shard to per-chunk column
sums and a 4KB AllGather completes the mean (fallback variant replicates
the full query read).

Inputs are staged transposed ([H, shard]) and in bf16 so the query
column-sums are free-axis DVE reduces and the g.key matvec is a chain of
bf16 PE matmuls accumulating in PSUM; 1/K is folded into Wg/bg and 1/S
into the qmean rows of Wa1 on the host.
"""

import os

import numpy as np

SEQ = 8192
H = 1024
F = 128
K = 2
NCORES = 8
SHARD = SEQ // NCORES  # 1024
NCH = H // 128  # 8 h-chunks of 128

# packed f32 constant tensor [128, NP_] column layout
_C_WT1 = 0
_C_BT1 = 1
_C_TS = 2
_C_WT2 = 4
_C_BT2 = 132
_C_DGT = 133
_C_BA1 = 135
_C_BA2 = 136
_C_BGT = 137
_C_WA2 = 145
_C_WA1M = 146
_C_WA1Q = 274
_C_WG = 274 + H
NP_ = _C_WG + H

_PROG_CACHE = {}


def _build(use_collective: bool):
    import concourse.bacc as bacc
    import concourse.tile as tile
    from concourse import mybir
    from concourse.tile_rust import add_dep_helper

    AF = mybir.ActivationFunctionType
    ALU = mybir.AluOpType
    f32 = mybir.dt.float32
    bf16 = mybir.dt.bfloat16

    nc = bacc.Bacc(
        "TRN2",
        target_bir_lowering=False,
        debug=False,
        num_devices=NCORES,
    )

    qcols = SHARD if use_collective else SEQ
    qs = nc.dram_tensor("qs", [H, qcols], bf16, kind="ExternalInput").ap()
    ks = nc.dram_tensor("ks", [H, SHARD], bf16, kind="ExternalInput").ap()
    Pc = nc.dram_tensor("P", [128, NP_], f32, kind="ExternalInput").ap()
    out = nc.dram_tensor("out", [1, SHARD], f32, kind="ExternalOutput").ap()

    with tile.TileContext(nc) as tc:
        with (
            tc.tile_pool(name="consts", bufs=1) as cp,
            tc.tile_pool(name="work", bufs=1) as wp,
            tc.tile_pool(name="qstream", bufs=NCH if use_collective else 3) as qp,
            tc.tile_pool(name="kstream", bufs=NCH) as kp,
            tc.tile_pool(name="ps_small", bufs=2, space="PSUM") as pps,
            tc.tile_pool(name="ps_keep", bufs=1, space="PSUM") as ppk,
            tc.tile_pool(name="ps_big", bufs=1, space="PSUM") as ppb,
            tc.tile_pool(name="dram", bufs=1, space="DRAM") as dp,
        ):
            # ---- query chunk loads get the wire first: they feed the
            # qmean partial sums and the collective, the head of the
            # critical path.  qtile[c][p, s] = q[s, c*128+p].
            qv = qs.rearrange("(c p) s -> c p s", p=128)
            qtiles, q_insts = [], []
            for c in range(NCH):
                qt = qp.tile([128, qcols], bf16, tag="qt")
                q_insts.append(nc.sync.dma_start(qt, qv[c]))
                qtiles.append(qt)

            # warm the ACT tables NOW against a dep-free memset tile: the
            # loads must not steal ACT time mid accum-chain or on the tail
            wsrc = wp.tile([1, 1], f32, tag="wsrc")
            nc.vector.memset(wsrc, 0.0)
            warm1 = wp.tile([1, 1], f32, tag="w1")
            nc.scalar.activation(warm1, wsrc, AF.Tanh)
            warm2 = wp.tile([1, 1], f32, tag="w2")
            nc.scalar.activation(warm2, wsrc, AF.Sigmoid)

            # ---- per-chunk query column sums, pipelined with the loads
            # and split across DVE (even chunks, reduce) and ACT (odd
            # chunks, copy+accum) so neither engine chain lags the
            # arriving tiles: qmTp[p, c] = sum_s q[s, c*128+p]
            qmTp = wp.tile([128, NCH], f32, tag="qmTp")
            junk = wp.tile([128, qcols], bf16, tag="junk")
            for c in range(NCH):
                if c % 2 == 0:
                    nc.vector.tensor_reduce(
                        qmTp[:, c : c + 1],
                        qtiles[c],
                        axis=mybir.AxisListType.X,
                        op=ALU.add,
                    )
                else:
                    nc.scalar.activation(
                        junk,
                        qtiles[c],
                        AF.Copy,
                        accum_out=qmTp[:, c : c + 1],
                    )

            cc_inst = None
            if use_collective:
                # gather in bf16: halves the collective's size-linear cost
                qmTp_bf = wp.tile([128, NCH], bf16, tag="qmTp_bf")
                nc.vector.tensor_copy(qmTp_bf, qmTp)
                cc_in = dp.tile([128, NCH], bf16)
                cc_out = dp.tile([NCORES, 128 * NCH], bf16)
                cc_inst = nc.scalar.dma_start(cc_in, qmTp_bf)
                nc.gpsimd.collective_compute(
                    "AllGather",
                    ALU.bypass,
                    replica_groups=[list(range(NCORES))],
                    ins=[cc_in.opt()],
                    outs=[cc_out.opt()],
                )

            # ---- packed constants: one DMA, kept off the wire until the
            # tiny collective input is out (it feeds only the k-side work
            # that hides under the collective)
            P = cp.tile([128, NP_], f32)
            p_inst = nc.scalar.dma_start(P, Pc)
            add_dep_helper(
                p_inst.ins,
                (cc_inst if cc_inst is not None else q_insts[-1]).ins,
                reason="consts after cc_in",
            )
            Wt1T = P[0:32, _C_WT1 : _C_WT1 + 1]
            bt1T = P[0:32, _C_BT1 : _C_BT1 + 1]
            tb = P[0:32, _C_TS : _C_TS + K]
            Wt2 = P[0:32, _C_WT2 : _C_WT2 + F]
            bt2T = P[:, _C_BT2 : _C_BT2 + 1]
            dgT = P[:, _C_DGT : _C_DGT + K]
            ba1T = P[:, _C_BA1 : _C_BA1 + 1]
            ba2c = P[0:K, _C_BA2 : _C_BA2 + 1]
            bgT = P[:, _C_BGT : _C_BGT + NCH]
            Wa2c = P[:, _C_WA2 : _C_WA2 + 1]
            Wa1m = P[:, _C_WA1M : _C_WA1M + F]

            # ---- key stream; ordered after cc_in so the tiny collective
            # input is not stuck behind 2MB of key reads
            kv = ks.rearrange("(c p) s -> c p s", p=128)
            ktiles = []
            for c in range(NCH):
                kt = kp.tile([128, SHARD], bf16, tag="kt")
                ki = nc.sync.dma_start(kt, kv[c])
                add_dep_helper(
                    ki.ins,
                    (cc_inst if cc_inst is not None else q_insts[-1]).ins,
                    reason="keys after cc_in",
                )
                ktiles.append(kt)

            # ---- temporal MLP -> memT [F, K] (f32) ----
            h1T = wp.tile([32, K], f32, tag="h1T")
            nc.vector.tensor_scalar_mul(h1T, tb, Wt1T)
            nc.vector.tensor_scalar_add(h1T, h1T, bt1T)
            nc.vector.tensor_relu(h1T, h1T)
            tT_ps = pps.tile([F, K], f32, tag="tmp")
            nc.tensor.matmul(tT_ps, lhsT=Wt2, rhs=h1T, start=True, stop=True)
            memT = wp.tile([F, K], f32, tag="memT")
            nc.scalar.activation(memT, tT_ps, AF.Identity, bias=bt2T, scale=1.0)
            nc.vector.tensor_add(memT, memT, dgT)

            # ---- mem half of the scorer (pre-collective) ----
            haT_ps = ppk.tile([F, K], f32, tag="haT")
            nc.tensor.matmul(haT_ps, lhsT=Wa1m, rhs=memT, start=True, stop=True)

            # ---- gate columns gT[c][p, k] = g_k[c*128+p] in bf16
            # (1/K and bg folded on the host)
            gt_bf = []
            for c in range(NCH):
                g_ps = pps.tile([F, K], f32, tag="gt")
                nc.tensor.matmul(
                    g_ps,
                    lhsT=P[:, _C_WG + c * 128 : _C_WG + (c + 1) * 128],
                    rhs=memT,
                    start=True,
                    stop=True,
                )
                gb = wp.tile([F, K], bf16, tag=f"gb{c}")
                nc.vector.tensor_scalar_add(gb, g_ps, bgT[:, c : c + 1])
                gt_bf.append(gb)

            # ---- matvec: row_ps[k, s] = g_k . key[s], chunked over h ----
            row_ps = ppb.tile([K, SHARD], f32, tag="big")
            for half in range(2):
                sl = slice(half * 512, (half + 1) * 512)
                for c in range(NCH):
                    nc.tensor.matmul(
                        row_ps[:, sl],
                        lhsT=gt_bf[c],
                        rhs=ktiles[c][:, sl],
                        start=(c == 0),
                        stop=(c == NCH - 1),
                    )
            row_sb = wp.tile([K, SHARD], bf16, tag="row")
            nc.scalar.copy(row_sb, row_ps)

            # keep PE ramped (pstate) through the collective window so the
            # post-collective matmuls run at full clock; tuned count ends
            # just before qmT becomes available
            n_dummy = int(os.environ.get("CA1_PE_WARM", "20")) if use_collective else 0
            if n_dummy:
                dmy_ps = ppb.tile([K, SHARD], f32, tag="big")
                for _ in range(n_dummy):
                    nc.tensor.matmul(
                        dmy_ps[:, 0:512],
                        lhsT=gt_bf[0],
                        rhs=ktiles[0][:, 0:512],
                        start=True,
                        stop=True,
                    )

            # ---- post-collective: qmT = sum of per-core partials ----
            if use_collective:
                qmTd8 = wp.tile([128, NCORES, NCH], bf16, tag="qmTd8")
                nc.sync.dma_start(
                    qmTd8, cc_out[:, :].rearrange("d (p c) -> p d c", c=NCH)
                )
                qmT = wp.tile([128, NCH], f32, tag="qmT")
                nc.vector.tensor_reduce(
                    qmT,
                    qmTd8.rearrange("p d c -> p c d"),
                    axis=mybir.AxisListType.X,
                    op=ALU.add,
                )
            else:
                qmT = qmTp

            # ---- hq[f] = (qmean @ Wa1q)[f]  (1/S folded into Wa1q) ----
            hq_ps = ppk.tile([F, 1], f32, tag="hq")
            for c in range(NCH):
                nc.tensor.matmul(
                    hq_ps,
                    lhsT=P[:, _C_WA1Q + c * 128 : _C_WA1Q + (c + 1) * 128],
                    rhs=qmT[:, c : c + 1],
                    start=(c == 0),
                    stop=(c == NCH - 1),
                )
            hq_sb = wp.tile([F, 1], f32, tag="hq_sb")
            nc.vector.tensor_scalar_add(hq_sb, hq_ps, ba1T)

            # ---- scorer tail: tanh, score, sigmoid ----
            aT = wp.tile([F, K], f32, tag="aT")
            nc.scalar.activation(aT, haT_ps, AF.Tanh, bias=hq_sb, scale=1.0)
            score_ps = pps.tile([K, 1], f32, tag="tmp")
            nc.tensor.matmul(score_ps, lhsT=aT, rhs=Wa2c, start=True, stop=True)
            wcol = wp.tile([K, 1], bf16, tag="wcol")
            nc.scalar.activation(wcol, score_ps, AF.Sigmoid, bias=ba2c, scale=1.0)

            # ---- combine anchors: o_row = wcol^T @ row  (1/K in row);
            # the PSUM->SBUF copy of each half overlaps the other half's
            # matmul (ACT vs PE)
            orow_ps = ppb.tile([K, SHARD], f32, tag="big")
            orow_sb = wp.tile([1, SHARD], f32, tag="orow")
            for half in range(2):
                sl = slice(half * 512, (half + 1) * 512)
                nc.tensor.matmul(
                    orow_ps[0:1, sl],
                    lhsT=wcol,
                    rhs=row_sb[:, sl],
                    start=True,
                    stop=True,
                )
                nc.scalar.copy(orow_sb[:, sl], orow_ps[0:1, sl])
                # per-half store: the first half's DMA overlaps the
                # second half's combine+copy
                nc.sync.dma_start(out[:, sl], orow_sb[:, sl])

    nc.compile()
    return nc


def _get_prog(use_collective: bool):
    key = bool(use_collective)
    if key not in _PROG_CACHE:
        _PROG_CACHE[key] = _build(key)
    return _PROG_CACHE[key]


def _pack_consts(inputs) -> np.ndarray:
    f = lambda name: np.asarray(inputs[name], np.float32)
    P = np.zeros((128, NP_), np.float32)
    P[0:32, _C_WT1] = f("Wt1")[0]
    P[0:32, _C_BT1] = f("bt1")
    P[0:32, _C_TS : _C_TS + K] = np.broadcast_to(f("timestamps")[None, :], (32, K))
    P[0:32, _C_WT2 : _C_WT2 + F] = f("Wt2")
    P[:, _C_BT2] = f("bt2")
    P[:, _C_DGT : _C_DGT + K] = f("dg_features").T
    P[:, _C_BA1] = f("ba1")
    P[0:K, _C_BA2] = f("ba2")[0]
    P[:, _C_BGT : _C_BGT + NCH] = (f("bg") * (1.0 / K)).reshape(NCH, 128).T
    P[:, _C_WA2] = f("Wa2")[:, 0]
    P[:, _C_WA1M : _C_WA1M + F] = f("Wa1")[0:F, :]
    P[:, _C_WA1Q : _C_WA1Q + H] = (
        (f("Wa1")[F:, :] * (1.0 / SEQ))
        .reshape(NCH, 128, F)
        .transpose(1, 0, 2)
        .reshape(128, NCH * F)
    )
    P[:, _C_WG : _C_WG + H] = f("Wg") * (1.0 / K)
    return np.ascontiguousarray(P)


def _make_in_maps(inputs, use_collective: bool):
    import ml_dtypes

    bf16 = ml_dtypes.bfloat16
    q = np.asarray(inputs["query"], np.float32)[0]  # [S,H]
    k = np.asarray(inputs["key"], np.float32)[0]  # [S,H]
    P = _pack_consts(inputs)
    qb = q.astype(bf16)
    kb = k.astype(bf16)
    if not use_collective:
        q_full = np.ascontiguousarray(qb.T)  # [H, S]
    in_maps = []
    for d in range(NCORES):
        sl = slice(d * SHARD, (d + 1) * SHARD)
        m = {
            "P": P,
            "ks": np.ascontiguousarray(kb[sl].T),  # [H, SHARD]
            "qs": (
                np.ascontiguousarray(qb[sl].T) if use_collective else q_full
            ),
        }
        in_maps.append(m)
    return in_maps


def _run(inputs, use_collective: bool, trace: bool = False):
    from concourse.bass_utils import run_bass_kernel_spmd

    nc = _get_prog(use_collective)
    in_maps = _make_in_maps(inputs, use_collective)
    res = run_bass_kernel_spmd(
        nc, in_maps, core_ids=list(range(NCORES)), trace=trace
    )
    row = np.empty((SEQ,), np.float32)
    for d in range(NCORES):
        row[d * SHARD : (d + 1) * SHARD] = res.results[d]["out"][0]
    # every row of the [S, S] output is the same gate row
    full = np.empty((1, 1, SEQ, SEQ), np.float32)
    full[0, 0, :, :] = row[None, :]
    return full, res


def kernel(**inputs) -> np.ndarray:
    use_collective = os.environ.get("CA1_NO_COLLECTIVE", "0") != "1"
    try:
        full, _ = _run(inputs, use_collective)
        return full
    except Exception:
        if not use_collective:
            raise
        # fall back to the zero-communication variant (replicated query)
        _PROG_CACHE.pop(True, None)
        full, _ = _run(inputs, False)
        return full


# revision 20
# speedup vs baseline: 1.0806x; 1.0105x over previous
"""Trainium2 Bass kernel for nn_CA1AttentionGate.

Computes, for full inputs (B=1, S=8192, H=1024, F=128, K=2):
    temporal = relu(t @ Wt1 + bt1) @ Wt2 + bt2          [K,F]
    mem      = dg_features + temporal                    [K,F]
    qmean    = query.mean(axis=1)                        [1,H]
    score_k  = tanh([mem_k ; qmean] @ Wa1 + ba1) @ Wa2 + ba2
    w_k      = sigmoid(score_k)
    g_k      = mem_k @ Wg + bg                           [K,H]
    row[s]   = (1/K) * sum_k w_k * (g_k . key[s])        [S]
    out      = broadcast(row) -> [1,1,S,S]

Sharding: sequence-parallel over the key/seq axis across 8 cores.  Every
row of the [S,S] output is the same vector, so each core computes only
its 1024-entry slice of that broadcast row from its key shard (the
sharding_hint's "slice of the broadcast row") and the host unshard step
expands the gathered row to the full output.  The only cross-core
quantity is qmean: each core reduces its query shard to per-chunk column
sums and a 4KB AllGather completes the mean (fallback variant replicates
the full query read).

Inputs are staged transposed ([H, shard]) and in bf16 so the query
column-sums are free-axis DVE reduces and the g.key matvec is a chain of
bf16 PE matmuls accumulating in PSUM; 1/K is folded into Wg/bg and 1/S
into the qmean rows of Wa1 on the host.
"""

import os

import numpy as np

SEQ = 8192
H = 1024
F = 128
K = 2
NCORES = 8
SHARD = SEQ // NCORES  # 1024
NCH = H // 128  # 8 h-chunks of 128

# packed f32 constant tensor [128, NP_] column layout
_C_WT1 = 0
_C_BT1 = 1
_C_TS = 2
_C_WT2 = 4
_C_BT2 = 132
_C_DGT = 133
_C_BA1 = 135
_C_BA2 = 136
_C_BGT = 137
_C_WA2 = 145
_C_WA1M = 146
_C_WA1Q = 274
_C_WG = 274 + H
NP_ = _C_WG + H

_PROG_CACHE = {}


def _build(use_collective: bool):
    import concourse.bacc as bacc
    import concourse.tile as tile
    from concourse import mybir
    from concourse.tile_rust import add_dep_helper

    AF = mybir.ActivationFunctionType
    ALU = mybir.AluOpType
    f32 = mybir.dt.float32
    bf16 = mybir.dt.bfloat16

    nc = bacc.Bacc(
        "TRN2",
        target_bir_lowering=False,
        debug=False,
        num_devices=NCORES,
    )

    qcols = SHARD if use_collective else SEQ
    qs = nc.dram_tensor("qs", [H, qcols], bf16, kind="ExternalInput").ap()
    ks = nc.dram_tensor("ks", [H, SHARD], bf16, kind="ExternalInput").ap()
    Pc = nc.dram_tensor("P", [128, NP_], f32, kind="ExternalInput").ap()
    out = nc.dram_tensor("out", [1, SHARD], f32, kind="ExternalOutput").ap()

    with tile.TileContext(nc) as tc:
        with (
            tc.tile_pool(name="consts", bufs=1) as cp,
            tc.tile_pool(name="work", bufs=1) as wp,
            tc.tile_pool(name="qstream", bufs=NCH if use_collective else 3) as qp,
            tc.tile_pool(name="kstream", bufs=NCH) as kp,
            tc.tile_pool(name="ps_small", bufs=2, space="PSUM") as pps,
            tc.tile_pool(name="ps_keep", bufs=1, space="PSUM") as ppk,
            tc.tile_pool(name="ps_big", bufs=1, space="PSUM") as ppb,
            tc.tile_pool(name="dram", bufs=1, space="DRAM") as dp,
        ):
            # ---- query chunk loads get the wire first: they feed the
            # qmean partial sums and the collective, the head of the
            # critical path.  qtile[c][p, s] = q[s, c*128+p].
            qv = qs.rearrange("(c p) s -> c p s", p=128)
            qtiles, q_insts = [], []
            for c in range(NCH):
                qt = qp.tile([128, qcols], bf16, tag="qt")
                q_insts.append(nc.sync.dma_start(qt, qv[c]))
                qtiles.append(qt)

            # warm the ACT tables NOW against a dep-free memset tile: the
            # loads must not steal ACT time mid accum-chain or on the tail
            wsrc = wp.tile([1, 1], f32, tag="wsrc")
            nc.vector.memset(wsrc, 0.0)
            warm1 = wp.tile([1, 1], f32, tag="w1")
            nc.scalar.activation(warm1, wsrc, AF.Tanh)
            warm2 = wp.tile([1, 1], f32, tag="w2")
            nc.scalar.activation(warm2, wsrc, AF.Sigmoid)

            # ---- per-chunk query column sums, pipelined with the loads
            # and split across DVE (even chunks, reduce) and ACT (odd
            # chunks, copy+accum) so neither engine chain lags the
            # arriving tiles: qmTp[p, c] = sum_s q[s, c*128+p]
            qmTp = wp.tile([128, NCH], f32, tag="qmTp")
            junk = wp.tile([128, qcols], bf16, tag="junk")
            for c in range(NCH):
                if c % 2 == 0:
                    nc.vector.tensor_reduce(
                        qmTp[:, c : c + 1],
                        qtiles[c],
                        axis=mybir.AxisListType.X,
                        op=ALU.add,
                    )
                else:
                    nc.scalar.activation(
                        junk,
                        qtiles[c],
                        AF.Copy,
                        accum_out=qmTp[:, c : c + 1],
                    )

            cc_inst = None
            if use_collective:
                # gather in bf16: halves the collective's size-linear cost
                # (SBUF-output collectives are rejected by the runtime, so
                # the gather stays a DRAM round trip)
                qmTp_bf = wp.tile([128, NCH], bf16, tag="qmTp_bf")
                nc.vector.tensor_copy(qmTp_bf, qmTp)
                cc_in = dp.tile([128, NCH], bf16)
                cc_out = dp.tile([NCORES, 128 * NCH], bf16)
                cc_inst = nc.scalar.dma_start(cc_in, qmTp_bf)
                nc.gpsimd.collective_compute(
                    "AllGather",
                    ALU.bypass,
                    replica_groups=[list(range(NCORES))],
                    ins=[cc_in.opt()],
                    outs=[cc_out.opt()],
                )

            # ---- packed constants: one DMA, kept off the wire until the
            # tiny collective input is out (it feeds only the k-side work
            # that hides under the collective)
            P = cp.tile([128, NP_], f32)
            p_inst = nc.scalar.dma_start(P, Pc)
            add_dep_helper(
                p_inst.ins,
                (cc_inst if cc_inst is not None else q_insts[-1]).ins,
                reason="consts after cc_in",
            )
            Wt1T = P[0:32, _C_WT1 : _C_WT1 + 1]
            bt1T = P[0:32, _C_BT1 : _C_BT1 + 1]
            tb = P[0:32, _C_TS : _C_TS + K]
            Wt2 = P[0:32, _C_WT2 : _C_WT2 + F]
            bt2T = P[:, _C_BT2 : _C_BT2 + 1]
            dgT = P[:, _C_DGT : _C_DGT + K]
            ba1T = P[:, _C_BA1 : _C_BA1 + 1]
            ba2c = P[0:K, _C_BA2 : _C_BA2 + 1]
            bgT = P[:, _C_BGT : _C_BGT + NCH]
            Wa2c = P[:, _C_WA2 : _C_WA2 + 1]
            Wa1m = P[:, _C_WA1M : _C_WA1M + F]

            # ---- key stream; ordered after cc_in so the tiny collective
            # input is not stuck behind 2MB of key reads
            kv = ks.rearrange("(c p) s -> c p s", p=128)
            ktiles = []
            for c in range(NCH):
                kt = kp.tile([128, SHARD], bf16, tag="kt")
                ki = nc.sync.dma_start(kt, kv[c])
                add_dep_helper(
                    ki.ins,
                    (cc_inst if cc_inst is not None else q_insts[-1]).ins,
                    reason="keys after cc_in",
                )
                ktiles.append(kt)

            # ---- temporal MLP -> memT [F, K] (f32) ----
            h1T = wp.tile([32, K], f32, tag="h1T")
            nc.vector.tensor_scalar_mul(h1T, tb, Wt1T)
            nc.vector.tensor_scalar_add(h1T, h1T, bt1T)
            nc.vector.tensor_relu(h1T, h1T)
            tT_ps = pps.tile([F, K], f32, tag="tmp")
            nc.tensor.matmul(tT_ps, lhsT=Wt2, rhs=h1T, start=True, stop=True)
            memT = wp.tile([F, K], f32, tag="memT")
            nc.scalar.activation(memT, tT_ps, AF.Identity, bias=bt2T, scale=1.0)
            nc.vector.tensor_add(memT, memT, dgT)

            # ---- mem half of the scorer (pre-collective) ----
            haT_ps = ppk.tile([F, K], f32, tag="haT")
            nc.tensor.matmul(haT_ps, lhsT=Wa1m, rhs=memT, start=True, stop=True)

            # ---- gate columns gT[c][p, k] = g_k[c*128+p] in bf16
            # (1/K and bg folded on the host)
            gt_bf = []
            for c in range(NCH):
                g_ps = pps.tile([F, K], f32, tag="gt")
                nc.tensor.matmul(
                    g_ps,
                    lhsT=P[:, _C_WG + c * 128 : _C_WG + (c + 1) * 128],
                    rhs=memT,
                    start=True,
                    stop=True,
                )
                gb = wp.tile([F, K], bf16, tag=f"gb{c}")
                nc.vector.tensor_scalar_add(gb, g_ps, bgT[:, c : c + 1])
                gt_bf.append(gb)

            # ---- matvec: row_ps[k, s] = g_k . key[s], chunked over h ----
            row_ps = ppb.tile([K, SHARD], f32, tag="big")
            for half in range(2):
                sl = slice(half * 512, (half + 1) * 512)
                for c in range(NCH):
                    nc.tensor.matmul(
                        row_ps[:, sl],
                        lhsT=gt_bf[c],
                        rhs=ktiles[c][:, sl],
                        start=(c == 0),
                        stop=(c == NCH - 1),
                    )
            row_sb = wp.tile([K, SHARD], bf16, tag="row")
            nc.scalar.copy(row_sb, row_ps)

            # keep PE ramped (pstate) through the collective window so the
            # post-collective matmuls run at full clock; tuned count ends
            # just before qmT becomes available
            n_dummy = int(os.environ.get("CA1_PE_WARM", "20")) if use_collective else 0
            if n_dummy:
                dmy_ps = ppb.tile([K, SHARD], f32, tag="big")
                for _ in range(n_dummy):
                    nc.tensor.matmul(
                        dmy_ps[:, 0:512],
                        lhsT=gt_bf[0],
                        rhs=ktiles[0][:, 0:512],
                        start=True,
                        stop=True,
                    )

            # ---- post-collective: qmT = sum of per-core partials ----
            if use_collective:
                qmTd8 = wp.tile([128, NCORES, NCH], bf16, tag="qmTd8")
                nc.sync.dma_start(
                    qmTd8, cc_out[:, :].rearrange("d (p c) -> p d c", c=NCH)
                )
                qmT = wp.tile([128, NCH], f32, tag="qmT")
                nc.vector.tensor_reduce(
                    qmT,
                    qmTd8.rearrange("p d c -> p c d"),
                    axis=mybir.AxisListType.X,
                    op=ALU.add,
                )
            else:
                qmT = qmTp

            # ---- hq[f] = (qmean @ Wa1q)[f]  (1/S folded into Wa1q) ----
            hq_ps = ppk.tile([F, 1], f32, tag="hq")
            for c in range(NCH):
                nc.tensor.matmul(
                    hq_ps,
                    lhsT=P[:, _C_WA1Q + c * 128 : _C_WA1Q + (c + 1) * 128],
                    rhs=qmT[:, c : c + 1],
                    start=(c == 0),
                    stop=(c == NCH - 1),
                )
            hq_sb = wp.tile([F, 1], f32, tag="hq_sb")
            nc.vector.tensor_scalar_add(hq_sb, hq_ps, ba1T)

            # ---- scorer tail: tanh, score, sigmoid ----
            aT = wp.tile([F, K], f32, tag="aT")
            nc.scalar.activation(aT, haT_ps, AF.Tanh, bias=hq_sb, scale=1.0)
            score_ps = pps.tile([K, 1], f32, tag="tmp")
            nc.tensor.matmul(score_ps, lhsT=aT, rhs=Wa2c, start=True, stop=True)
            wcol = wp.tile([K, 1], bf16, tag="wcol")
            nc.scalar.activation(wcol, score_ps, AF.Sigmoid, bias=ba2c, scale=1.0)

            # ---- combine anchors: o_row = wcol^T @ row  (1/K in row);
            # the PSUM->SBUF copy of each half overlaps the other half's
            # matmul (ACT vs PE)
            orow_ps = ppb.tile([K, SHARD], f32, tag="big")
            orow_sb = wp.tile([1, SHARD], f32, tag="orow")
            for half in range(2):
                sl = slice(half * 512, (half + 1) * 512)
                nc.tensor.matmul(
                    orow_ps[0:1, sl],
                    lhsT=wcol,
                    rhs=row_sb[:, sl],
                    start=True,
                    stop=True,
                )
                # PSUM->SBUF copies split across ACT and DVE so the two
                # halves overlap each other and the second matmul
                if half == 0:
                    nc.scalar.copy(orow_sb[:, sl], orow_ps[0:1, sl])
                else:
                    nc.vector.tensor_copy(orow_sb[:, sl], orow_ps[0:1, sl])
            nc.sync.dma_start(out, orow_sb)

    nc.compile()
    return nc


def _get_prog(use_collective: bool):
    key = bool(use_collective)
    if key not in _PROG_CACHE:
        _PROG_CACHE[key] = _build(key)
    return _PROG_CACHE[key]


def _pack_consts(inputs) -> np.ndarray:
    f = lambda name: np.asarray(inputs[name], np.float32)
    P = np.zeros((128, NP_), np.float32)
    P[0:32, _C_WT1] = f("Wt1")[0]
    P[0:32, _C_BT1] = f("bt1")
    P[0:32, _C_TS : _C_TS + K] = np.broadcast_to(f("timestamps")[None, :], (32, K))
    P[0:32, _C_WT2 : _C_WT2 + F] = f("Wt2")
    P[:, _C_BT2] = f("bt2")
    P[:, _C_DGT : _C_DGT + K] = f("dg_features").T
    P[:, _C_BA1] = f("ba1")
    P[0:K, _C_BA2] = f("ba2")[0]
    P[:, _C_BGT : _C_BGT + NCH] = (f("bg") * (1.0 / K)).reshape(NCH, 128).T
    P[:, _C_WA2] = f("Wa2")[:, 0]
    P[:, _C_WA1M : _C_WA1M + F] = f("Wa1")[0:F, :]
    P[:, _C_WA1Q : _C_WA1Q + H] = (
        (f("Wa1")[F:, :] * (1.0 / SEQ))
        .reshape(NCH, 128, F)
        .transpose(1, 0, 2)
        .reshape(128, NCH * F)
    )
    P[:, _C_WG : _C_WG + H] = f("Wg") * (1.0 / K)
    return np.ascontiguousarray(P)


def _make_in_maps(inputs, use_collective: bool):
    import ml_dtypes

    bf16 = ml_dtypes.bfloat16
    q = np.asarray(inputs["query"], np.float32)[0]  # [S,H]
    k = np.asarray(inputs["key"], np.float32)[0]  # [S,H]
    P = _pack_consts(inputs)
    qb = q.astype(bf16)
    kb = k.astype(bf16)
    if not use_collective:
        q_full = np.ascontiguousarray(qb.T)  # [H, S]
    in_maps = []
    for d in range(NCORES):
        sl = slice(d * SHARD, (d + 1) * SHARD)
        m = {
            "P": P,
            "ks": np.ascontiguousarray(kb[sl].T),  # [H, SHARD]
            "qs": (
                np.ascontiguousarray(qb[sl].T) if use_collective else q_full
            ),
        }
        in_maps.append(m)
    return in_maps


def _run(inputs, use_collective: bool, trace: bool = False):
    from concourse.bass_utils import run_bass_kernel_spmd

    nc = _get_prog(use_collective)
    in_maps = _make_in_maps(inputs, use_collective)
    res = run_bass_kernel_spmd(
        nc, in_maps, core_ids=list(range(NCORES)), trace=trace
    )
    row = np.empty((SEQ,), np.float32)
    for d in range(NCORES):
        row[d * SHARD : (d + 1) * SHARD] = res.results[d]["out"][0]
    # every row of the [S, S] output is the same gate row
    full = np.empty((1, 1, SEQ, SEQ), np.float32)
    full[0, 0, :, :] = row[None, :]
    return full, res


def kernel(**inputs) -> np.ndarray:
    use_collective = os.environ.get("CA1_NO_COLLECTIVE", "0") != "1"
    try:
        full, _ = _run(inputs, use_collective)
        return full
    except Exception:
        if not use_collective:
            raise
        # fall back to the zero-communication variant (replicated query)
        _PROG_CACHE.pop(True, None)
        full, _ = _run(inputs, False)
        return full
